# revision 54
# baseline (speedup 1.0000x reference)
"""ContextBottleneck kernel for 8 TRN2 NeuronCores (fp8 DoubleRow matmuls,
PE-computed mean, residual via PE identity-matmul, scale-folded drain).

Data-parallel over the 16384 tokens (2048 tokens/core). Host-side prep
(layout/constant folding only; all token-dependent math stays on-chip):
  - alpha*b_up/(1-alpha) folded into hidden (h_aug).
  - hs16 = fp16(2*h_aug): residual + LN-variance input (fp16 keeps the
    residual path accurate to ~2e-4 rms; LN is scale-invariant so the
    factor cancels; 2 keeps bn_stats M2 partials inside fp16 range).
  - hT8  = fp8e4(h_aug), transposed host-side (d on partitions), grouped
    per 512 tokens with k-chunk pairs adjacent for DoubleRow.
  - wd8  = fp8e4(16*gamma*W_down), wu8 = fp8e4((2048*alpha/(1-alpha))*W_up).
  - Residual identity is 1024*I in fp16 (exact pow2); the final drain
    out = k*psum with k = (1-alpha)/2048 then gives
    k*(1024*hs16 + s@wu8) = (1-alpha)*h_aug + alpha*(s@W_up).
Per core pipeline (normalize folded AFTER matmul1):
  mean: Sum(x) row via DoubleRow ones-matmul over hT8 into a psum row,
    -mu16 row = one DVE scaled psum-read (no partition flip needed)
  variance: bn_stats/bn_aggr on fp16 hs16 -> quake rsqrt of
    (var16*(S/ch)^2 + S^2*eps) [= rsig/S] -> DVE 32x32 stream-transpose
    flip to a row -> gpsimd partition_broadcast
  -> mm1: fp8 DoubleRow over 16 d-chunks (raw hT8, unnormalized)
     + K=1 bf16 matmul q x (-mu16) (mean correction, q = S/ch*colsum)
  -> DVE: z = psum * rsig/S broadcast  [bf16]
  -> ACT: s = Silu(z + b_down) -> fp8e4
  -> mm2: fp8 DoubleRow (s stationary, wu8 moving)
     + fp16 1024*identity matmul accumulating hs16 into the same psum
  -> drain: out = k*psum (pure scaled ACT copy) -> store f32.
PE stream is software-pipelined ([DRs(g+1)] [mm2(g)] [K1s(g+1)]) so the
in-order PE queue never stalls on the silu or stats dependencies; DMA is
spread over all three DGE queues (SP: hs16, ACT: wd/q, SWDGE: ht8 +
weights + stores) because one queue only streams ~170GB/s and a blocked
dispatch stalls the issuing engine's whole FIFO.
"""

import numpy as np
import ml_dtypes

import concourse.bacc as bacc
import concourse.tile as tile
from concourse import mybir
from concourse.bass_utils import run_bass_kernel_spmd

AF = mybir.ActivationFunctionType
ALU = mybir.AluOpType
PM = mybir.MatmulPerfMode
BF16 = mybir.dt.bfloat16
F16 = mybir.dt.float16
F32 = mybir.dt.float32
I32 = mybir.dt.int32
FP8E4 = mybir.dt.float8e4

D = 2048
DB = 512
N_CORES = 8
KD = D // 128     # 16 contraction chunks for matmul1
KB = DB // 128    # 4 bottleneck chunks
NCOL = D // 512   # 4 output column chunks
LN_EPS = 1e-5
WD_SCALE = 16.0   # wd8 = fp8(16*Wd)
CH = 2.0          # hs16 = fp16(2*h_aug): keeps bn_stats M2 in fp16 range
CI = 1024.0       # residual identity is CI*I (exact pow2); the drain scale
                  # k = (1-alpha)/(CI*CH) then sets the wu8 scale to a
                  # comfortably-normal e4m3 range


def build_kernel(T, alpha, act_func=None):
    act_func = AF.Silu if act_func is None else act_func
    nc = bacc.Bacc(
        "TRN2",
        target_bir_lowering=False,
        debug=False,
        enable_asserts=True,
        num_devices=N_CORES,
    )
    n_groups = T // 512
    assert T % 512 == 0

    h_d = nc.dram_tensor("h", [T, D], F16, kind="ExternalInput").ap()
    ht_d = nc.dram_tensor("ht", [n_groups * 128, KD * 512], FP8E4,
                          kind="ExternalInput").ap()
    wd_d = nc.dram_tensor("wd", [128, KD * DB], FP8E4, kind="ExternalInput").ap()
    wu_d = nc.dram_tensor("wu", [128, KB * D], FP8E4, kind="ExternalInput").ap()
    q_d = nc.dram_tensor("qs", [1, DB], BF16, kind="ExternalInput").ap()
    b1_d = nc.dram_tensor("b1", [128, KB], F32, kind="ExternalInput").ap()
    id_d = nc.dram_tensor("ident", [128, 128], F16, kind="ExternalInput").ap()
    o_d = nc.dram_tensor("o", [T, D], F32, kind="ExternalOutput").ap()

    drain_k = (1.0 - alpha) / (CI * CH)
    rs_mult = (WD_SCALE / CH) ** 2
    rs_add = WD_SCALE * WD_SCALE * LN_EPS

    with tile.TileContext(nc) as tc:
        with (
            tc.tile_pool(name="singles", bufs=1) as singles,
            tc.tile_pool(name="hp", bufs=8) as h_pool,
            tc.tile_pool(name="htp", bufs=3) as ht_pool,
            tc.tile_pool(name="zsp", bufs=6) as zs_pool,
            tc.tile_pool(name="sp", bufs=3) as s_pool,
            tc.tile_pool(name="resp", bufs=3) as res_pool,
            tc.tile_pool(name="stp", bufs=4) as st_pool,
            tc.tile_pool(name="rowp", bufs=3) as row_pool,
            tc.tile_pool(name="zpp", bufs=4, space="PSUM") as zp_pool,
            tc.tile_pool(name="opp", bufs=4, space="PSUM") as op_pool,
        ):
            # Ring plan (a blocked DMA dispatch stalls the whole strict-FIFO
            # engine queue behind it, and each DGE queue only streams
            # ~170GB/s, so spread + keep compute engines' queues clean):
            #   scalar HWDGE: wd + q only, then ACT is pure compute
            #   sync HWDGE:   the 16 hs16 tiles (SP has no compute)
            #   gpsimd SWDGE: ht8 group tiles, wu/b1/id, output stores
            wd_sb = singles.tile([128, KD, DB], FP8E4)
            nc.scalar.dma_start(wd_sb[:], wd_d[:])
            q_sb = singles.tile([1, DB], BF16)
            nc.scalar.dma_start(q_sb[:], q_d[:])
            wu_sb = singles.tile([128, KB, D], FP8E4)
            b1_sb = singles.tile([128, KB], F32)
            id_sb = singles.tile([128, 128], F16)

            # DR stationary of ones for the Sum(x) mean matmul; inner dim
            # padded to 16B so the DoubleRow middle-dim step is 16-aligned
            ones_sb = singles.tile([128, KD, 16], FP8E4)
            nc.vector.memset(ones_sb[:], 1.0)
            c_nm = -CH / D  # -mu16 row from Sum(x) over unit-scale ht8

            def emit_early_singles():
                nc.gpsimd.dma_start(b1_sb[:], b1_d[:])
                nc.gpsimd.dma_start(id_sb[:], id_d[:])

            def emit_late_singles():
                nc.gpsimd.dma_start(wu_sb[:], wu_d[:])

            def emit_rsqrt_flip(mvg_j, nmrs, rows_nm, rows_rs, js):
                """rsqrt of a = rs_mult*var + rs_add for tiles `js`, then flip
                -mu16 / (rsig/S) to rows at partition 0 via DVE 32x32 stream
                transposes. mvg_j is [128, n, 2] (n = len(js))."""
                n = len(js)
                a = st_pool.tile([128, n], F32, tag=f"qa{n}")
                nc.vector.tensor_scalar(
                    a[:], mvg_j[:, :, 1], rs_mult, rs_add, ALU.mult, ALU.add
                )
                ya = st_pool.tile([128, n], F32, tag=f"qya{n}")
                yb = st_pool.tile([128, n], F32, tag=f"qyb{n}")
                t1 = st_pool.tile([128, n], F32, tag=f"qt1{n}")
                t2 = st_pool.tile([128, n], F32, tag=f"qt2{n}")
                nc.vector.tensor_scalar(
                    t1[:].bitcast(I32),
                    a[:].bitcast(I32),
                    1,
                    -1,
                    ALU.logical_shift_right,
                    ALU.bitwise_xor,
                )
                nc.vector.tensor_scalar(
                    ya[:].bitcast(I32),
                    t1[:].bitcast(I32),
                    0x5F3759E0,
                    None,
                    ALU.add,
                )
                cur, nxt = ya, yb
                for _ in range(2):
                    nc.vector.tensor_mul(t1[:], cur[:], cur[:])
                    nc.vector.scalar_tensor_tensor(
                        t2[:], t1[:], -0.5, a[:], ALU.mult, ALU.mult
                    )
                    nc.vector.scalar_tensor_tensor(
                        nxt[:], t2[:], 1.5, cur[:], ALU.add, ALU.mult
                    )
                    cur, nxt = nxt, cur
                # nmrs col 1 = rsig/S (col 0 unused: the -mu row now comes
                # from the PE Sum(x) matmul, no flip needed)
                nc.vector.tensor_copy(nmrs[:, js[0] : js[0] + n, 1], cur[:])
                for j in js:
                    for b in range(4):
                        dst = slice(j * 128 + b * 32, j * 128 + b * 32 + 32)
                        nc.vector.transpose(
                            rows_rs[0:32, dst],
                            nmrs[b * 32 : b * 32 + 32, j, 1:33],
                        )

            def emit_stats(g, h_tiles):
                """bn stats (variance only) + rsqrt + rsig row flip.
                Group 0 runs per-tile so the first z-scale deps are ready as
                soon as each hs16 tile lands (shorter pipeline fill)."""
                per_tile = g == 0
                mvg = st_pool.tile([128, 4, 2], F32, tag="mvg")
                nmrs = st_pool.tile([128, 4, 34], F32, tag="nmrs")
                rows_nm = None
                rows_rs = row_pool.tile([32, 512], F32, tag="rowsrs")
                rsb = row_pool.tile([128, 512], F32, tag="rsb")
                with tc.high_priority():
                    nc.vector.memset(nmrs[:], 0.0)
                for j in range(4):
                    st6 = st_pool.tile([128, 4, 6], F16, tag="st6")
                    for sub in range(4):
                        nc.vector.bn_stats(
                            st6[:, sub, :],
                            h_tiles[j][:, sub * 512 : (sub + 1) * 512],
                        )
                    nc.vector.bn_aggr(mvg[:, j, :], st6[:])
                    if per_tile:
                        with tc.high_priority():
                            emit_rsqrt_flip(
                                mvg[:, j : j + 1, :], nmrs, rows_nm, rows_rs,
                                [j],
                            )
                            nc.gpsimd.partition_broadcast(
                                rsb[:, j * 128 : (j + 1) * 128],
                                rows_rs[0:1, j * 128 : (j + 1) * 128],
                            )
                if not per_tile:
                    with tc.high_priority():
                        emit_rsqrt_flip(mvg[:], nmrs, rows_nm, rows_rs,
                                        [0, 1, 2, 3])
                        nc.gpsimd.partition_broadcast(rsb[:], rows_rs[0:1, :])
                return rsb

            def emit_group(g):
                h_tiles = []
                ht8 = ht_pool.tile([128, KD, 512], FP8E4, tag="ht8")
                nc.gpsimd.dma_start(ht8[:], ht_d[g * 128 : (g + 1) * 128, :])
                for j in range(4):
                    ht = h_pool.tile([128, D], F16, tag="ht")
                    row0 = (g * 4 + j) * 128
                    nc.sync.dma_start(ht[:], h_d[row0 : row0 + 128, :])
                    h_tiles.append(ht)
                rsb = emit_stats(g, h_tiles)
                return h_tiles, ht8, rsb

            def emit_mm1_dr(g, st):
                """Sum(x) mean matmul + matmul1 fp8-DR bulk (depends only on
                wd + ht8). The -mu row is a single DVE psum-read away."""
                ht8 = st[1]
                # Sum(x) accumulates into row 0 of the first zp bank as its
                # own psum group; the DR group below then resets the bank.
                zp0 = zp_pool.tile([128, 512], F32, tag="zp")
                for c in range(KD // 2):
                    nc.tensor.matmul(
                        zp0[0:1, :],
                        ones_sb[:, 2 * c : 2 * c + 2, 0:1],
                        ht8[:, 2 * c : 2 * c + 2, :],
                        start=(c == 0),
                        stop=(c == KD // 2 - 1),
                        perf_mode=PM.DoubleRow,
                    )
                nm_bf = row_pool.tile([1, 512], BF16, tag="nmbf")
                with tc.high_priority():
                    nc.vector.tensor_scalar(
                        nm_bf[:], zp0[0:1, :], c_nm, None, ALU.mult
                    )
                zps = []
                for db in range(KB):
                    zp = zp0 if db == 0 else zp_pool.tile(
                        [128, 512], F32, tag="zp"
                    )
                    for c in range(KD // 2):
                        nc.tensor.matmul(
                            zp[:],
                            wd_sb[:, 2 * c : 2 * c + 2, db * 128 : (db + 1) * 128],
                            ht8[:, 2 * c : 2 * c + 2, :],
                            start=(c == 0),
                            stop=False,
                            perf_mode=PM.DoubleRow,
                        )
                    zps.append(zp)
                return zps, nm_bf

            def emit_mm1_fix(g, st, mm1):
                """K=1 mean fix -> z-scale -> silu."""
                h_tiles, ht8, rsb = st
                zps, nm_bf = mm1
                for db in range(KB):
                    nc.tensor.matmul(
                        zps[db][:],
                        q_sb[0:1, db * 128 : (db + 1) * 128],
                        nm_bf[0:1, :],
                        start=False,
                        stop=True,
                    )
                # z-scale + silu at high priority: the next group's bulk
                # bn_stats must not queue ahead of them on DVE/ACT (mm2 of
                # this group hangs off silu)
                sg = s_pool.tile([128, KB, 512], FP8E4, tag="sg")
                with tc.high_priority():
                    for db in range(KB):
                        zs = zs_pool.tile([128, 512], BF16, tag="zs")
                        nc.vector.tensor_tensor(
                            zs[:], zps[db][:], rsb[:], ALU.mult
                        )
                        nc.scalar.activation(
                            sg[:, db, :],
                            zs[:],
                            act_func,
                            bias=b1_sb[:, db : db + 1],
                            scale=1.0,
                        )
                return sg

            def emit_mm2(g, st, sg):
                """matmul2 (fp8 DR) + residual identity-matmul + drain."""
                h_tiles = st[0]
                for j in range(4):
                    ops = []
                    for dcol in range(NCOL):
                        op_t = op_pool.tile([128, 512], F32, tag="op_t")
                        ops.append(op_t)
                        for k in range(KB // 2):
                            nc.tensor.matmul(
                                op_t[:],
                                sg[:, 2 * k : 2 * k + 2, j * 128 : (j + 1) * 128],
                                wu_sb[:, 2 * k : 2 * k + 2,
                                      dcol * 512 : (dcol + 1) * 512],
                                start=(k == 0),
                                stop=False,
                                perf_mode=PM.DoubleRow,
                            )
                        # residual: psum += I @ hs16 (fp16, exact identity)
                        nc.tensor.matmul(
                            op_t[:],
                            id_sb[:],
                            h_tiles[j][:, dcol * 512 : (dcol + 1) * 512],
                            start=False,
                            stop=True,
                        )
                    res = res_pool.tile([128, D], F32, tag="res")
                    # drain: out = k*psum (pure scaled copy on ACT; putting
                    # any of these on DVE delays the z-scale chain and loses
                    # ~7us overall)
                    with tc.high_priority():
                        for dcol in range(NCOL):
                            dst = res[:, dcol * 512 : (dcol + 1) * 512]
                            nc.scalar.mul(dst, ops[dcol][:], drain_k)
                    row0 = (g * 4 + j) * 128
                    # the last group's stores ride the (by then idle) SP
                    # ring instead of queueing behind 12 stores on SWDGE
                    st_eng = nc.sync if g == n_groups - 1 else nc.gpsimd
                    st_eng.dma_start(o_d[row0 : row0 + 128, :], res[:])

            # Software-pipelined emission. PE FIFO order per group:
            #   [DRs(g+1)] [mm2(g)] [K1s(g+1)]
            # so the stats-gated K=1s of g+1 sit BEHIND mm2(g) in the strict
            # in-order PE queue: mm2(g) never blocks on stats(g+1), and the
            # K=1s get mm2(g)'s ~6us as slack for the stats chain to finish.
            st = [emit_group(0)]
            emit_early_singles()
            st.append(emit_group(1) if n_groups > 1 else None)
            zps0 = emit_mm1_dr(0, st[0])
            sg = [emit_mm1_fix(0, st[0], zps0)]
            zps_nxt = emit_mm1_dr(1, st[1]) if n_groups > 1 else None
            for g in range(n_groups):
                if g + 2 < n_groups:
                    st.append(emit_group(g + 2))
                else:
                    st.append(None)
                if g == 0:
                    # wu/b1/id ride SWDGE *behind* ht8(2): ht8 gates mm1 of
                    # its group, the weights aren't needed until mm2(0)
                    emit_late_singles()
                emit_mm2(g, st[g], sg[g])
                if g + 1 < n_groups:
                    sg.append(emit_mm1_fix(g + 1, st[g + 1], zps_nxt))
                    zps_nxt = emit_mm1_dr(g + 2, st[g + 2]) if g + 2 < n_groups else None
                else:
                    sg.append(None)

    nc.compile()
    return nc


def prep_host_inputs(hidden, ln_gamma, ln_beta, W_down, b_down, W_up, b_up, alpha):
    bf = ml_dtypes.bfloat16
    e4 = ml_dtypes.float8_e4m3
    hidden = np.asarray(hidden, np.float32)
    gam = np.asarray(ln_gamma, np.float32)
    bet = np.asarray(ln_beta, np.float32)
    Wd = np.asarray(W_down, np.float32)
    bd = np.asarray(b_down, np.float32)
    Wu = np.asarray(W_up, np.float32)
    bu = np.asarray(b_up, np.float32)
    alpha = float(alpha)

    # fold alpha*b_up/(1-alpha) into hidden
    h_aug = hidden.reshape(-1, D) + (alpha / (1.0 - alpha)) * bu[None, :]
    hs16 = (CH * h_aug).astype(np.float16)  # [T_full, D] row layout
    T = h_aug.shape[0] // N_CORES
    n_groups = T // 512
    ht8 = np.ascontiguousarray(
        h_aug.astype(e4)
        .reshape(N_CORES, n_groups, 512, KD, 128)
        .transpose(0, 1, 4, 3, 2)
        .reshape(N_CORES, n_groups * 128, KD * 512)
    )

    Wdg = gam[:, None] * Wd  # [D, DB]
    wd8 = np.ascontiguousarray(
        (WD_SCALE * Wdg).astype(e4)
        .reshape(KD, 128, DB).transpose(1, 0, 2).reshape(128, KD * DB)
    )
    q_h = np.ascontiguousarray(
        ((WD_SCALE / CH) * Wdg.sum(axis=0)).astype(bf).reshape(1, DB)
    )
    b1_h = np.ascontiguousarray(
        (bet @ Wdg + bd).astype(np.float32).reshape(KB, 128).T
    )  # [128, KB]
    cw = CI * CH * alpha / (1.0 - alpha)
    wu8 = np.ascontiguousarray(
        (cw * Wu).astype(e4)
        .reshape(KB, 128, D).transpose(1, 0, 2).reshape(128, KB * D)
    )
    ident = (CI * np.eye(128)).astype(np.float16)
    return hs16, ht8, wd8, wu8, q_h, b1_h, ident, alpha


_cached = {}


def kernel(
    hidden,
    ln_gamma,
    ln_beta,
    W_down,
    b_down,
    W_up,
    b_up,
    alpha,
    layer_idx=None,
    **_unused,
):
    hs16, ht8, wd8, wu8, q_h, b1_h, ident, alpha_f = prep_host_inputs(
        hidden, ln_gamma, ln_beta, W_down, b_down, W_up, b_up, alpha
    )
    T = hs16.shape[0] // N_CORES
    key = (T, alpha_f)
    if key not in _cached:
        _cached[key] = build_kernel(T, alpha_f)
    nc = _cached[key]

    h_shards = hs16.reshape(N_CORES, T, D)
    in_maps = [
        {
            "h": np.ascontiguousarray(h_shards[c]),
            "ht": ht8[c],
            "wd": wd8,
            "wu": wu8,
            "qs": q_h,
            "b1": b1_h,
            "ident": ident,
        }
        for c in range(N_CORES)
    ]
    res = run_bass_kernel_spmd(nc, in_maps, list(range(N_CORES)))
    global _last_results
    _last_results = res
    out = np.concatenate([r["o"] for r in res.results], axis=0)
    return out.reshape(np.asarray(hidden).shape).astype(np.float32)


_last_results = None


# revision 55
# speedup vs baseline: 1.1244x; 1.1244x over previous
"""ContextBottleneck kernel for 8 TRN2 NeuronCores (fp8 DoubleRow matmuls,
PE-computed mean, residual via PE identity-matmul, scale-folded drain).

Data-parallel over the 16384 tokens (2048 tokens/core). Host-side prep
(layout/constant folding only; all token-dependent math stays on-chip):
  - alpha*b_up/(1-alpha) folded into hidden (h_aug).
  - hs16 = fp16(2*h_aug): residual + LN-variance input (fp16 keeps the
    residual path accurate to ~2e-4 rms; LN is scale-invariant so the
    factor cancels; 2 keeps bn_stats M2 partials inside fp16 range).
  - hT8  = fp8e4(h_aug), transposed host-side (d on partitions), grouped
    per 512 tokens with k-chunk pairs adjacent for DoubleRow.
  - wd8  = fp8e4(16*gamma*W_down), wu8 = fp8e4((2048*alpha/(1-alpha))*W_up).
  - Residual identity is 1024*I in fp16 (exact pow2); the final drain
    out = k*psum with k = (1-alpha)/2048 then gives
    k*(1024*hs16 + s@wu8) = (1-alpha)*h_aug + alpha*(s@W_up).
Per core pipeline (normalize folded AFTER matmul1):
  mean: Sum(x) row via DoubleRow ones-matmul over hT8 into a psum row,
    -mu16 row = one DVE scaled psum-read (no partition flip needed)
  variance: bn_stats/bn_aggr on fp16 hs16 -> quake rsqrt of
    (var16*(S/ch)^2 + S^2*eps) [= rsig/S] -> DVE 32x32 stream-transpose
    flip to a row -> gpsimd partition_broadcast
  -> mm1: fp8 DoubleRow over 16 d-chunks (raw hT8, unnormalized)
     + K=1 bf16 matmul q x (-mu16) (mean correction, q = S/ch*colsum)
  -> DVE: z = psum * rsig/S broadcast  [bf16]
  -> ACT: s = Silu(z + b_down) -> fp8e4
  -> mm2: fp8 DoubleRow (s stationary, wu8 moving)
     + fp16 1024*identity matmul accumulating hs16 into the same psum
  -> drain: out = k*psum (pure scaled ACT copy) -> store f32.
PE stream is software-pipelined ([DRs(g+1)] [mm2(g)] [K1s(g+1)]) so the
in-order PE queue never stalls on the silu or stats dependencies; DMA is
spread over all three DGE queues (SP: hs16, ACT: wd/q, SWDGE: ht8 +
weights + stores) because one queue only streams ~170GB/s and a blocked
dispatch stalls the issuing engine's whole FIFO.
"""

import numpy as np
import ml_dtypes

import concourse.bacc as bacc
import concourse.tile as tile
from concourse import mybir
from concourse.bass_utils import run_bass_kernel_spmd

AF = mybir.ActivationFunctionType
ALU = mybir.AluOpType
PM = mybir.MatmulPerfMode
BF16 = mybir.dt.bfloat16
F16 = mybir.dt.float16
F32 = mybir.dt.float32
I32 = mybir.dt.int32
FP8E4 = mybir.dt.float8e4

D = 2048
DB = 512
N_CORES = 8
KD = D // 128     # 16 contraction chunks for matmul1
KB = DB // 128    # 4 bottleneck chunks
NCOL = D // 512   # 4 output column chunks
LN_EPS = 1e-5
WD_SCALE = 16.0   # wd8 = fp8(16*Wd)
CH = 2.0          # hs16 = fp16(2*h_aug): keeps bn_stats M2 in fp16 range
CI = 1024.0       # residual identity is CI*I (exact pow2); the drain scale
                  # k = (1-alpha)/(CI*CH) then sets the wu8 scale to a
                  # comfortably-normal e4m3 range


def build_kernel(T, alpha, act_func=None):
    act_func = AF.Silu if act_func is None else act_func
    nc = bacc.Bacc(
        "TRN2",
        target_bir_lowering=False,
        debug=False,
        enable_asserts=True,
        num_devices=N_CORES,
    )
    n_groups = T // 512
    assert T % 512 == 0

    h_d = nc.dram_tensor("h", [T, D], F16, kind="ExternalInput").ap()
    ht_d = nc.dram_tensor("ht", [n_groups * 128, KD * 512], FP8E4,
                          kind="ExternalInput").ap()
    wd_d = nc.dram_tensor("wd", [128, KD * DB], FP8E4, kind="ExternalInput").ap()
    wu_d = nc.dram_tensor("wu", [128, KB * D], FP8E4, kind="ExternalInput").ap()
    q_d = nc.dram_tensor("qs", [1, DB], BF16, kind="ExternalInput").ap()
    b1_d = nc.dram_tensor("b1", [128, KB], F32, kind="ExternalInput").ap()
    id_d = nc.dram_tensor("ident", [128, 128], F16, kind="ExternalInput").ap()
    o_d = nc.dram_tensor("o", [T, D], F32, kind="ExternalOutput").ap()

    drain_k = (1.0 - alpha) / (CI * CH)
    rs_mult = (WD_SCALE / CH) ** 2
    rs_add = WD_SCALE * WD_SCALE * LN_EPS

    with tile.TileContext(nc) as tc:
        with (
            tc.tile_pool(name="singles", bufs=1) as singles,
            tc.tile_pool(name="hp", bufs=8) as h_pool,
            tc.tile_pool(name="htp", bufs=3) as ht_pool,
            tc.tile_pool(name="zsp", bufs=6) as zs_pool,
            tc.tile_pool(name="sp", bufs=3) as s_pool,
            tc.tile_pool(name="resp", bufs=3) as res_pool,
            tc.tile_pool(name="stp", bufs=4) as st_pool,
            tc.tile_pool(name="rowp", bufs=3) as row_pool,
            tc.tile_pool(name="zpp", bufs=4, space="PSUM") as zp_pool,
            tc.tile_pool(name="opp", bufs=4, space="PSUM") as op_pool,
        ):
            # Ring plan (a blocked DMA dispatch stalls the whole strict-FIFO
            # engine queue behind it, and each DGE queue only streams
            # ~170GB/s, so spread + keep compute engines' queues clean):
            #   scalar HWDGE: wd + q only, then ACT is pure compute
            #   sync HWDGE:   the 16 hs16 tiles (SP has no compute)
            #   gpsimd SWDGE: ht8 group tiles, wu/b1/id, output stores
            wd_sb = singles.tile([128, KD, DB], FP8E4)
            nc.scalar.dma_start(wd_sb[:], wd_d[:])
            q_sb = singles.tile([1, DB], BF16)
            nc.scalar.dma_start(q_sb[:], q_d[:])
            wu_sb = singles.tile([128, KB, D], FP8E4)
            b1_sb = singles.tile([128, KB], F32)
            id_sb = singles.tile([128, 128], F16)

            # DR stationary of ones for the Sum(x) mean matmul; inner dim
            # padded to 16B so the DoubleRow middle-dim step is 16-aligned
            ones_sb = singles.tile([128, KD, 16], FP8E4)
            nc.vector.memset(ones_sb[:], 1.0)
            c_nm = -CH / D  # -mu16 row from Sum(x) over unit-scale ht8

            def emit_early_singles():
                nc.gpsimd.dma_start(b1_sb[:], b1_d[:])
                nc.gpsimd.dma_start(id_sb[:], id_d[:])

            def emit_late_singles():
                nc.gpsimd.dma_start(wu_sb[:], wu_d[:])

            def emit_rsqrt_flip(mvg_j, nmrs, rows_nm, rows_rs, js):
                """rsqrt of a = rs_mult*var + rs_add for tiles `js`, then flip
                -mu16 / (rsig/S) to rows at partition 0 via DVE 32x32 stream
                transposes. mvg_j is [128, n, 2] (n = len(js))."""
                n = len(js)
                a = st_pool.tile([128, n], F32, tag=f"qa{n}")
                nc.vector.tensor_scalar(
                    a[:], mvg_j[:, :, 1], rs_mult, rs_add, ALU.mult, ALU.add
                )
                ya = st_pool.tile([128, n], F32, tag=f"qya{n}")
                yb = st_pool.tile([128, n], F32, tag=f"qyb{n}")
                t1 = st_pool.tile([128, n], F32, tag=f"qt1{n}")
                t2 = st_pool.tile([128, n], F32, tag=f"qt2{n}")
                nc.vector.tensor_scalar(
                    t1[:].bitcast(I32),
                    a[:].bitcast(I32),
                    1,
                    -1,
                    ALU.logical_shift_right,
                    ALU.bitwise_xor,
                )
                nc.vector.tensor_scalar(
                    ya[:].bitcast(I32),
                    t1[:].bitcast(I32),
                    0x5F3759E0,
                    None,
                    ALU.add,
                )
                cur, nxt = ya, yb
                for _ in range(2):
                    nc.vector.tensor_mul(t1[:], cur[:], cur[:])
                    nc.vector.scalar_tensor_tensor(
                        t2[:], t1[:], -0.5, a[:], ALU.mult, ALU.mult
                    )
                    nc.vector.scalar_tensor_tensor(
                        nxt[:], t2[:], 1.5, cur[:], ALU.add, ALU.mult
                    )
                    cur, nxt = nxt, cur
                # nmrs col 1 = rsig/S (col 0 unused: the -mu row now comes
                # from the PE Sum(x) matmul, no flip needed)
                nc.vector.tensor_copy(nmrs[:, js[0] : js[0] + n, 1], cur[:])
                for j in js:
                    for b in range(4):
                        dst = slice(j * 128 + b * 32, j * 128 + b * 32 + 32)
                        nc.vector.transpose(
                            rows_rs[0:32, dst],
                            nmrs[b * 32 : b * 32 + 32, j, 1:33],
                        )

            def emit_stats(g, h_tiles):
                """bn stats (variance only) + rsqrt + rsig row flip.
                Group 0 runs per-tile so the first z-scale deps are ready as
                soon as each hs16 tile lands (shorter pipeline fill)."""
                per_tile = g == 0
                mvg = st_pool.tile([128, 4, 2], F32, tag="mvg")
                nmrs = st_pool.tile([128, 4, 34], F32, tag="nmrs")
                rows_nm = None
                rows_rs = row_pool.tile([32, 512], F32, tag="rowsrs")
                rsb = row_pool.tile([128, 512], F32, tag="rsb")
                with tc.high_priority():
                    nc.vector.memset(nmrs[:], 0.0)
                for j in range(4):
                    st6 = st_pool.tile([128, 4, 6], F16, tag="st6")
                    for sub in range(4):
                        nc.vector.bn_stats(
                            st6[:, sub, :],
                            h_tiles[j][:, sub * 512 : (sub + 1) * 512],
                        )
                    nc.vector.bn_aggr(mvg[:, j, :], st6[:])
                    if per_tile:
                        with tc.high_priority():
                            emit_rsqrt_flip(
                                mvg[:, j : j + 1, :], nmrs, rows_nm, rows_rs,
                                [j],
                            )
                            nc.gpsimd.partition_broadcast(
                                rsb[:, j * 128 : (j + 1) * 128],
                                rows_rs[0:1, j * 128 : (j + 1) * 128],
                            )
                if not per_tile:
                    with tc.high_priority():
                        emit_rsqrt_flip(mvg[:], nmrs, rows_nm, rows_rs,
                                        [0, 1, 2, 3])
                        nc.gpsimd.partition_broadcast(rsb[:], rows_rs[0:1, :])
                return rsb

            def emit_group(g):
                h_tiles = []
                ht8 = ht_pool.tile([128, KD, 512], FP8E4, tag="ht8")
                nc.gpsimd.dma_start(ht8[:], ht_d[g * 128 : (g + 1) * 128, :])
                for j in range(4):
                    ht = h_pool.tile([128, D], F16, tag="ht")
                    row0 = (g * 4 + j) * 128
                    nc.sync.dma_start(ht[:], h_d[row0 : row0 + 128, :])
                    h_tiles.append(ht)
                rsb = emit_stats(g, h_tiles)
                return h_tiles, ht8, rsb

            def emit_mm1_dr(g, st):
                """Sum(x) mean matmul + matmul1 fp8-DR bulk (depends only on
                wd + ht8). The -mu row is a single DVE psum-read away."""
                ht8 = st[1]
                # Sum(x) accumulates into row 0 of the first zp bank as its
                # own psum group; the DR group below then resets the bank.
                zp0 = zp_pool.tile([128, 512], F32, tag="zp")
                for c in range(KD // 2):
                    nc.tensor.matmul(
                        zp0[0:1, :],
                        ones_sb[:, 2 * c : 2 * c + 2, 0:1],
                        ht8[:, 2 * c : 2 * c + 2, :],
                        start=(c == 0),
                        stop=(c == KD // 2 - 1),
                        perf_mode=PM.DoubleRow,
                    )
                nm_bf = row_pool.tile([1, 512], BF16, tag="nmbf")
                with tc.high_priority():
                    nc.vector.tensor_scalar(
                        nm_bf[:], zp0[0:1, :], c_nm, None, ALU.mult
                    )
                zps = []
                for db in range(KB):
                    zp = zp0 if db == 0 else zp_pool.tile(
                        [128, 512], F32, tag="zp"
                    )
                    for c in range(KD // 2):
                        nc.tensor.matmul(
                            zp[:],
                            wd_sb[:, 2 * c : 2 * c + 2, db * 128 : (db + 1) * 128],
                            ht8[:, 2 * c : 2 * c + 2, :],
                            start=(c == 0),
                            stop=False,
                            perf_mode=PM.DoubleRow,
                        )
                    zps.append(zp)
                return zps, nm_bf

            def emit_mm1_fix(g, st, mm1):
                """K=1 mean fix -> z-scale -> silu."""
                h_tiles, ht8, rsb = st
                zps, nm_bf = mm1
                for db in range(KB):
                    nc.tensor.matmul(
                        zps[db][:],
                        q_sb[0:1, db * 128 : (db + 1) * 128],
                        nm_bf[0:1, :],
                        start=False,
                        stop=True,
                    )
                # z-scale + silu at high priority: the next group's bulk
                # bn_stats must not queue ahead of them on DVE/ACT (mm2 of
                # this group hangs off silu)
                sg = s_pool.tile([128, KB, 512], FP8E4, tag="sg")
                with tc.high_priority():
                    for db in range(KB):
                        zs = zs_pool.tile([128, 512], BF16, tag="zs")
                        nc.vector.tensor_tensor(
                            zs[:], zps[db][:], rsb[:], ALU.mult
                        )
                        nc.scalar.activation(
                            sg[:, db, :],
                            zs[:],
                            act_func,
                            bias=b1_sb[:, db : db + 1],
                            scale=1.0,
                        )
                return sg

            def emit_mm2(g, st, sg):
                """matmul2 (fp8 DR) + residual identity-matmul + drain."""
                h_tiles = st[0]
                for j in range(4):
                    ops = []
                    for dcol in range(NCOL):
                        op_t = op_pool.tile([128, 512], F32, tag="op_t")
                        ops.append(op_t)
                        for k in range(KB // 2):
                            nc.tensor.matmul(
                                op_t[:],
                                sg[:, 2 * k : 2 * k + 2, j * 128 : (j + 1) * 128],
                                wu_sb[:, 2 * k : 2 * k + 2,
                                      dcol * 512 : (dcol + 1) * 512],
                                start=(k == 0),
                                stop=False,
                                perf_mode=PM.DoubleRow,
                            )
                        # residual: psum += I @ hs16 (fp16, exact identity)
                        nc.tensor.matmul(
                            op_t[:],
                            id_sb[:],
                            h_tiles[j][:, dcol * 512 : (dcol + 1) * 512],
                            start=False,
                            stop=True,
                        )
                    res = res_pool.tile([128, D], F32, tag="res")
                    # drain: out = k*psum (pure scaled copy on ACT; putting
                    # any of these on DVE delays the z-scale chain and loses
                    # ~7us overall)
                    with tc.high_priority():
                        for dcol in range(NCOL):
                            dst = res[:, dcol * 512 : (dcol + 1) * 512]
                            nc.scalar.mul(dst, ops[dcol][:], drain_k)
                    row0 = (g * 4 + j) * 128
                    # all stores on SWDGE: routing any of them to the HWDGE
                    # rings (SP or scalar) was measured 1-20us slower --
                    # their dispatch stalls behind those rings' load queues
                    nc.gpsimd.dma_start(o_d[row0 : row0 + 128, :], res[:])

            # Software-pipelined emission. PE FIFO order per group:
            #   [DRs(g+1)] [mm2(g)] [K1s(g+1)]
            # so the stats-gated K=1s of g+1 sit BEHIND mm2(g) in the strict
            # in-order PE queue: mm2(g) never blocks on stats(g+1), and the
            # K=1s get mm2(g)'s ~6us as slack for the stats chain to finish.
            st = [emit_group(0)]
            emit_early_singles()
            st.append(emit_group(1) if n_groups > 1 else None)
            zps0 = emit_mm1_dr(0, st[0])
            sg = [emit_mm1_fix(0, st[0], zps0)]
            zps_nxt = emit_mm1_dr(1, st[1]) if n_groups > 1 else None
            for g in range(n_groups):
                if g + 2 < n_groups:
                    st.append(emit_group(g + 2))
                else:
                    st.append(None)
                if g == 0:
                    # wu/b1/id ride SWDGE *behind* ht8(2): ht8 gates mm1 of
                    # its group, the weights aren't needed until mm2(0)
                    emit_late_singles()
                emit_mm2(g, st[g], sg[g])
                if g + 1 < n_groups:
                    sg.append(emit_mm1_fix(g + 1, st[g + 1], zps_nxt))
                    zps_nxt = emit_mm1_dr(g + 2, st[g + 2]) if g + 2 < n_groups else None
                else:
                    sg.append(None)

    nc.compile()
    return nc


def prep_host_inputs(hidden, ln_gamma, ln_beta, W_down, b_down, W_up, b_up, alpha):
    bf = ml_dtypes.bfloat16
    e4 = ml_dtypes.float8_e4m3
    hidden = np.asarray(hidden, np.float32)
    gam = np.asarray(ln_gamma, np.float32)
    bet = np.asarray(ln_beta, np.float32)
    Wd = np.asarray(W_down, np.float32)
    bd = np.asarray(b_down, np.float32)
    Wu = np.asarray(W_up, np.float32)
    bu = np.asarray(b_up, np.float32)
    alpha = float(alpha)

    # fold alpha*b_up/(1-alpha) into hidden
    h_aug = hidden.reshape(-1, D) + (alpha / (1.0 - alpha)) * bu[None, :]
    hs16 = (CH * h_aug).astype(np.float16)  # [T_full, D] row layout
    T = h_aug.shape[0] // N_CORES
    n_groups = T // 512
    ht8 = np.ascontiguousarray(
        h_aug.astype(e4)
        .reshape(N_CORES, n_groups, 512, KD, 128)
        .transpose(0, 1, 4, 3, 2)
        .reshape(N_CORES, n_groups * 128, KD * 512)
    )

    Wdg = gam[:, None] * Wd  # [D, DB]
    wd8 = np.ascontiguousarray(
        (WD_SCALE * Wdg).astype(e4)
        .reshape(KD, 128, DB).transpose(1, 0, 2).reshape(128, KD * DB)
    )
    q_h = np.ascontiguousarray(
        ((WD_SCALE / CH) * Wdg.sum(axis=0)).astype(bf).reshape(1, DB)
    )
    b1_h = np.ascontiguousarray(
        (bet @ Wdg + bd).astype(np.float32).reshape(KB, 128).T
    )  # [128, KB]
    cw = CI * CH * alpha / (1.0 - alpha)
    wu8 = np.ascontiguousarray(
        (cw * Wu).astype(e4)
        .reshape(KB, 128, D).transpose(1, 0, 2).reshape(128, KB * D)
    )
    ident = (CI * np.eye(128)).astype(np.float16)
    return hs16, ht8, wd8, wu8, q_h, b1_h, ident, alpha


_cached = {}


def kernel(
    hidden,
    ln_gamma,
    ln_beta,
    W_down,
    b_down,
    W_up,
    b_up,
    alpha,
    layer_idx=None,
    **_unused,
):
    hs16, ht8, wd8, wu8, q_h, b1_h, ident, alpha_f = prep_host_inputs(
        hidden, ln_gamma, ln_beta, W_down, b_down, W_up, b_up, alpha
    )
    T = hs16.shape[0] // N_CORES
    key = (T, alpha_f)
    if key not in _cached:
        _cached[key] = build_kernel(T, alpha_f)
    nc = _cached[key]

    h_shards = hs16.reshape(N_CORES, T, D)
    in_maps = [
        {
            "h": np.ascontiguousarray(h_shards[c]),
            "ht": ht8[c],
            "wd": wd8,
            "wu": wu8,
            "qs": q_h,
            "b1": b1_h,
            "ident": ident,
        }
        for c in range(N_CORES)
    ]
    res = run_bass_kernel_spmd(nc, in_maps, list(range(N_CORES)))
    global _last_results
    _last_results = res
    out = np.concatenate([r["o"] for r in res.results], axis=0)
    return out.reshape(np.asarray(hidden).shape).astype(np.float32)


_last_results = None


# revision 58
# speedup vs baseline: 1.1290x; 1.0040x over previous
"""ContextBottleneck kernel for 8 TRN2 NeuronCores (fp8 DoubleRow matmuls,
PE-computed mean, residual via PE identity-matmul, scale-folded drain).

Data-parallel over the 16384 tokens (2048 tokens/core). Host-side prep
(layout/constant folding only; all token-dependent math stays on-chip):
  - alpha*b_up/(1-alpha) folded into hidden (h_aug).
  - hs16 = fp16(2*h_aug): residual + LN-variance input (fp16 keeps the
    residual path accurate to ~2e-4 rms; LN is scale-invariant so the
    factor cancels; 2 keeps bn_stats M2 partials inside fp16 range).
  - hT8  = fp8e4(h_aug), transposed host-side (d on partitions), grouped
    per 512 tokens with k-chunk pairs adjacent for DoubleRow.
  - wd8  = fp8e4(16*gamma*W_down), wu8 = fp8e4((2048*alpha/(1-alpha))*W_up).
  - Residual identity is 1024*I in fp16 (exact pow2); the final drain
    out = k*psum with k = (1-alpha)/2048 then gives
    k*(1024*hs16 + s@wu8) = (1-alpha)*h_aug + alpha*(s@W_up).
Per core pipeline (normalize folded AFTER matmul1):
  mean: Sum(x) row via DoubleRow ones-matmul over hT8 into a psum row,
    -mu16 row = one DVE scaled psum-read (no partition flip needed)
  variance: bn_stats/bn_aggr on fp16 hs16 -> quake rsqrt of
    (var16*(S/ch)^2 + S^2*eps) [= rsig/S] -> DVE 32x32 stream-transpose
    flip to a row -> gpsimd partition_broadcast
  -> mm1: fp8 DoubleRow over 16 d-chunks (raw hT8, unnormalized)
     + K=1 bf16 matmul q x (-mu16) (mean correction, q = S/ch*colsum)
  -> DVE: z = psum * rsig/S broadcast  [bf16]
  -> ACT: s = Silu(z + b_down) -> fp8e4
  -> mm2: fp8 DoubleRow (s stationary, wu8 moving)
     + fp16 1024*identity matmul accumulating hs16 into the same psum
  -> drain: out = k*psum (pure scaled ACT copy) -> store f32.
PE stream is software-pipelined ([DRs(g+1)] [mm2(g)] [K1s(g+1)]) so the
in-order PE queue never stalls on the silu or stats dependencies; DMA is
spread over all three DGE queues (SP: hs16, ACT: wd/q, SWDGE: ht8 +
weights + stores) because one queue only streams ~170GB/s and a blocked
dispatch stalls the issuing engine's whole FIFO.
"""

import numpy as np
import ml_dtypes

import concourse.bacc as bacc
import concourse.tile as tile
from concourse import mybir
from concourse.bass_utils import run_bass_kernel_spmd

AF = mybir.ActivationFunctionType
ALU = mybir.AluOpType
PM = mybir.MatmulPerfMode
BF16 = mybir.dt.bfloat16
F16 = mybir.dt.float16
F32 = mybir.dt.float32
I32 = mybir.dt.int32
FP8E4 = mybir.dt.float8e4

D = 2048
DB = 512
N_CORES = 8
KD = D // 128     # 16 contraction chunks for matmul1
KB = DB // 128    # 4 bottleneck chunks
NCOL = D // 512   # 4 output column chunks
LN_EPS = 1e-5
WD_SCALE = 16.0   # wd8 = fp8(16*Wd)
CH = 2.0          # hs16 = fp16(2*h_aug): keeps bn_stats M2 in fp16 range
CI = 1024.0       # residual identity is CI*I (exact pow2); the drain scale
                  # k = (1-alpha)/(CI*CH) then sets the wu8 scale to a
                  # comfortably-normal e4m3 range


def build_kernel(T, alpha, act_func=None):
    act_func = AF.Silu if act_func is None else act_func
    nc = bacc.Bacc(
        "TRN2",
        target_bir_lowering=False,
        debug=False,
        enable_asserts=True,
        num_devices=N_CORES,
    )
    n_groups = T // 512
    assert T % 512 == 0

    h_d = nc.dram_tensor("h", [T, D], F16, kind="ExternalInput").ap()
    ht_d = nc.dram_tensor("ht", [n_groups * 128, KD * 512], FP8E4,
                          kind="ExternalInput").ap()
    wd_d = nc.dram_tensor("wd", [128, KD * DB], FP8E4, kind="ExternalInput").ap()
    wu_d = nc.dram_tensor("wu", [128, KB * D], FP8E4, kind="ExternalInput").ap()
    q_d = nc.dram_tensor("qs", [1, DB], BF16, kind="ExternalInput").ap()
    b1_d = nc.dram_tensor("b1", [128, KB], F32, kind="ExternalInput").ap()
    id_d = nc.dram_tensor("ident", [128, 128], F16, kind="ExternalInput").ap()
    o_d = nc.dram_tensor("o", [T, D], F32, kind="ExternalOutput").ap()

    drain_k = (1.0 - alpha) / (CI * CH)
    rs_mult = (WD_SCALE / CH) ** 2
    rs_add = WD_SCALE * WD_SCALE * LN_EPS

    with tile.TileContext(nc) as tc:
        with (
            tc.tile_pool(name="singles", bufs=1) as singles,
            tc.tile_pool(name="hp", bufs=8) as h_pool,
            tc.tile_pool(name="htp", bufs=3) as ht_pool,
            tc.tile_pool(name="zsp", bufs=6) as zs_pool,
            tc.tile_pool(name="sp", bufs=3) as s_pool,
            tc.tile_pool(name="resp", bufs=3) as res_pool,
            tc.tile_pool(name="stp", bufs=4) as st_pool,
            tc.tile_pool(name="rowp", bufs=3) as row_pool,
            tc.tile_pool(name="zpp", bufs=4, space="PSUM") as zp_pool,
            tc.tile_pool(name="opp", bufs=4, space="PSUM") as op_pool,
        ):
            # Ring plan (a blocked DMA dispatch stalls the whole strict-FIFO
            # engine queue behind it, and each DGE queue only streams
            # ~170GB/s, so spread + keep compute engines' queues clean):
            #   scalar HWDGE: wd + q only, then ACT is pure compute
            #   sync HWDGE:   the 16 hs16 tiles (SP has no compute)
            #   gpsimd SWDGE: ht8 group tiles, wu/b1/id, output stores
            wd_sb = singles.tile([128, KD, DB], FP8E4)
            nc.scalar.dma_start(wd_sb[:], wd_d[:])
            q_sb = singles.tile([1, DB], BF16)
            nc.scalar.dma_start(q_sb[:], q_d[:])
            wu_sb = singles.tile([128, KB, D], FP8E4)
            b1_sb = singles.tile([128, KB], F32)
            id_sb = singles.tile([128, 128], F16)

            # DR stationary of ones for the Sum(x) mean matmul; inner dim
            # padded to 16B so the DoubleRow middle-dim step is 16-aligned
            ones_sb = singles.tile([128, KD, 16], FP8E4)
            nc.vector.memset(ones_sb[:], 1.0)
            c_nm = -CH / D  # -mu16 row from Sum(x) over unit-scale ht8

            def emit_late_singles():
                nc.gpsimd.dma_start(wu_sb[:], wu_d[:])
                nc.gpsimd.dma_start(b1_sb[:], b1_d[:])
                nc.gpsimd.dma_start(id_sb[:], id_d[:])

            def emit_rsqrt_flip(mvg_j, nmrs, rows_nm, rows_rs, js):
                """rsqrt of a = rs_mult*var + rs_add for tiles `js`, then flip
                -mu16 / (rsig/S) to rows at partition 0 via DVE 32x32 stream
                transposes. mvg_j is [128, n, 2] (n = len(js))."""
                n = len(js)
                a = st_pool.tile([128, n], F32, tag=f"qa{n}")
                nc.vector.tensor_scalar(
                    a[:], mvg_j[:, :, 1], rs_mult, rs_add, ALU.mult, ALU.add
                )
                ya = st_pool.tile([128, n], F32, tag=f"qya{n}")
                yb = st_pool.tile([128, n], F32, tag=f"qyb{n}")
                t1 = st_pool.tile([128, n], F32, tag=f"qt1{n}")
                t2 = st_pool.tile([128, n], F32, tag=f"qt2{n}")
                nc.vector.tensor_scalar(
                    t1[:].bitcast(I32),
                    a[:].bitcast(I32),
                    1,
                    -1,
                    ALU.logical_shift_right,
                    ALU.bitwise_xor,
                )
                nc.vector.tensor_scalar(
                    ya[:].bitcast(I32),
                    t1[:].bitcast(I32),
                    0x5F3759E0,
                    None,
                    ALU.add,
                )
                cur, nxt = ya, yb
                for _ in range(2):
                    nc.vector.tensor_mul(t1[:], cur[:], cur[:])
                    nc.vector.scalar_tensor_tensor(
                        t2[:], t1[:], -0.5, a[:], ALU.mult, ALU.mult
                    )
                    nc.vector.scalar_tensor_tensor(
                        nxt[:], t2[:], 1.5, cur[:], ALU.add, ALU.mult
                    )
                    cur, nxt = nxt, cur
                # nmrs col 1 = rsig/S (col 0 unused: the -mu row now comes
                # from the PE Sum(x) matmul, no flip needed)
                nc.vector.tensor_copy(nmrs[:, js[0] : js[0] + n, 1], cur[:])
                for j in js:
                    for b in range(4):
                        dst = slice(j * 128 + b * 32, j * 128 + b * 32 + 32)
                        nc.vector.transpose(
                            rows_rs[0:32, dst],
                            nmrs[b * 32 : b * 32 + 32, j, 1:33],
                        )

            def emit_stats(g, h_tiles):
                """bn stats (variance only) + rsqrt + rsig row flip.
                Group 0 runs per-tile so the first z-scale deps are ready as
                soon as each hs16 tile lands (shorter pipeline fill)."""
                per_tile = g == 0
                mvg = st_pool.tile([128, 4, 2], F32, tag="mvg")
                nmrs = st_pool.tile([128, 4, 34], F32, tag="nmrs")
                rows_nm = None
                rows_rs = row_pool.tile([32, 512], F32, tag="rowsrs")
                rsb = row_pool.tile([128, 512], F32, tag="rsb")
                with tc.high_priority():
                    nc.vector.memset(nmrs[:], 0.0)
                for j in range(4):
                    st6 = st_pool.tile([128, 4, 6], F16, tag="st6")
                    for sub in range(4):
                        nc.vector.bn_stats(
                            st6[:, sub, :],
                            h_tiles[j][:, sub * 512 : (sub + 1) * 512],
                        )
                    nc.vector.bn_aggr(mvg[:, j, :], st6[:])
                    if per_tile:
                        with tc.high_priority():
                            emit_rsqrt_flip(
                                mvg[:, j : j + 1, :], nmrs, rows_nm, rows_rs,
                                [j],
                            )
                            nc.gpsimd.partition_broadcast(
                                rsb[:, j * 128 : (j + 1) * 128],
                                rows_rs[0:1, j * 128 : (j + 1) * 128],
                            )
                if not per_tile:
                    with tc.high_priority():
                        emit_rsqrt_flip(mvg[:], nmrs, rows_nm, rows_rs,
                                        [0, 1, 2, 3])
                        nc.gpsimd.partition_broadcast(rsb[:], rows_rs[0:1, :])
                return rsb

            def emit_group(g):
                h_tiles = []
                ht8 = ht_pool.tile([128, KD, 512], FP8E4, tag="ht8")
                nc.gpsimd.dma_start(ht8[:], ht_d[g * 128 : (g + 1) * 128, :])
                for j in range(4):
                    ht = h_pool.tile([128, D], F16, tag="ht")
                    row0 = (g * 4 + j) * 128
                    nc.sync.dma_start(ht[:], h_d[row0 : row0 + 128, :])
                    h_tiles.append(ht)
                rsb = emit_stats(g, h_tiles)
                return h_tiles, ht8, rsb

            def emit_mm1_dr(g, st):
                """Sum(x) mean matmul + matmul1 fp8-DR bulk (depends only on
                wd + ht8). The -mu row is a single DVE psum-read away."""
                ht8 = st[1]
                # Sum(x) accumulates into row 0 of the first zp bank as its
                # own psum group; the DR group below then resets the bank.
                zp0 = zp_pool.tile([128, 512], F32, tag="zp")
                for c in range(KD // 2):
                    nc.tensor.matmul(
                        zp0[0:1, :],
                        ones_sb[:, 2 * c : 2 * c + 2, 0:1],
                        ht8[:, 2 * c : 2 * c + 2, :],
                        start=(c == 0),
                        stop=(c == KD // 2 - 1),
                        perf_mode=PM.DoubleRow,
                    )
                nm_bf = row_pool.tile([1, 512], BF16, tag="nmbf")
                with tc.high_priority():
                    nc.vector.tensor_scalar(
                        nm_bf[:], zp0[0:1, :], c_nm, None, ALU.mult
                    )
                zps = []
                for db in range(KB):
                    zp = zp0 if db == 0 else zp_pool.tile(
                        [128, 512], F32, tag="zp"
                    )
                    for c in range(KD // 2):
                        nc.tensor.matmul(
                            zp[:],
                            wd_sb[:, 2 * c : 2 * c + 2, db * 128 : (db + 1) * 128],
                            ht8[:, 2 * c : 2 * c + 2, :],
                            start=(c == 0),
                            stop=False,
                            perf_mode=PM.DoubleRow,
                        )
                    zps.append(zp)
                return zps, nm_bf

            def emit_mm1_fix(g, st, mm1):
                """K=1 mean fix -> z-scale -> silu."""
                h_tiles, ht8, rsb = st
                zps, nm_bf = mm1
                for db in range(KB):
                    nc.tensor.matmul(
                        zps[db][:],
                        q_sb[0:1, db * 128 : (db + 1) * 128],
                        nm_bf[0:1, :],
                        start=False,
                        stop=True,
                    )
                # z-scale + silu at high priority: the next group's bulk
                # bn_stats must not queue ahead of them on DVE/ACT (mm2 of
                # this group hangs off silu)
                sg = s_pool.tile([128, KB, 512], FP8E4, tag="sg")
                with tc.high_priority():
                    for db in range(KB):
                        zs = zs_pool.tile([128, 512], BF16, tag="zs")
                        nc.vector.tensor_tensor(
                            zs[:], zps[db][:], rsb[:], ALU.mult
                        )
                        nc.scalar.activation(
                            sg[:, db, :],
                            zs[:],
                            act_func,
                            bias=b1_sb[:, db : db + 1],
                            scale=1.0,
                        )
                return sg

            def emit_mm2(g, st, sg):
                """matmul2 (fp8 DR) + residual identity-matmul + drain."""
                h_tiles = st[0]
                for j in range(4):
                    ops = []
                    for dcol in range(NCOL):
                        op_t = op_pool.tile([128, 512], F32, tag="op_t")
                        ops.append(op_t)
                        for k in range(KB // 2):
                            nc.tensor.matmul(
                                op_t[:],
                                sg[:, 2 * k : 2 * k + 2, j * 128 : (j + 1) * 128],
                                wu_sb[:, 2 * k : 2 * k + 2,
                                      dcol * 512 : (dcol + 1) * 512],
                                start=(k == 0),
                                stop=False,
                                perf_mode=PM.DoubleRow,
                            )
                        # residual: psum += I @ hs16 (fp16, exact identity)
                        nc.tensor.matmul(
                            op_t[:],
                            id_sb[:],
                            h_tiles[j][:, dcol * 512 : (dcol + 1) * 512],
                            start=False,
                            stop=True,
                        )
                    res = res_pool.tile([128, D], F32, tag="res")
                    # drain: out = k*psum (pure scaled copy on ACT; putting
                    # any of these on DVE delays the z-scale chain and loses
                    # ~7us overall)
                    with tc.high_priority():
                        for dcol in range(NCOL):
                            dst = res[:, dcol * 512 : (dcol + 1) * 512]
                            nc.scalar.mul(dst, ops[dcol][:], drain_k)
                    row0 = (g * 4 + j) * 128
                    # all stores on SWDGE: routing any of them to the HWDGE
                    # rings (SP or scalar) was measured 1-20us slower --
                    # their dispatch stalls behind those rings' load queues
                    nc.gpsimd.dma_start(o_d[row0 : row0 + 128, :], res[:])

            # Software-pipelined emission. PE FIFO order per group:
            #   [DRs(g+1)] [mm2(g)] [K1s(g+1)]
            # so the stats-gated K=1s of g+1 sit BEHIND mm2(g) in the strict
            # in-order PE queue: mm2(g) never blocks on stats(g+1), and the
            # K=1s get mm2(g)'s ~6us as slack for the stats chain to finish.
            st = [emit_group(0)]
            emit_late_singles()
            st.append(emit_group(1) if n_groups > 1 else None)
            zps0 = emit_mm1_dr(0, st[0])
            sg = [emit_mm1_fix(0, st[0], zps0)]
            zps_nxt = emit_mm1_dr(1, st[1]) if n_groups > 1 else None
            for g in range(n_groups):
                if g + 2 < n_groups:
                    st.append(emit_group(g + 2))
                else:
                    st.append(None)
                emit_mm2(g, st[g], sg[g])
                if g + 1 < n_groups:
                    sg.append(emit_mm1_fix(g + 1, st[g + 1], zps_nxt))
                    zps_nxt = emit_mm1_dr(g + 2, st[g + 2]) if g + 2 < n_groups else None
                else:
                    sg.append(None)

    nc.compile()
    return nc


def prep_host_inputs(hidden, ln_gamma, ln_beta, W_down, b_down, W_up, b_up, alpha):
    bf = ml_dtypes.bfloat16
    e4 = ml_dtypes.float8_e4m3
    hidden = np.asarray(hidden, np.float32)
    gam = np.asarray(ln_gamma, np.float32)
    bet = np.asarray(ln_beta, np.float32)
    Wd = np.asarray(W_down, np.float32)
    bd = np.asarray(b_down, np.float32)
    Wu = np.asarray(W_up, np.float32)
    bu = np.asarray(b_up, np.float32)
    alpha = float(alpha)

    # fold alpha*b_up/(1-alpha) into hidden
    h_aug = hidden.reshape(-1, D) + (alpha / (1.0 - alpha)) * bu[None, :]
    hs16 = (CH * h_aug).astype(np.float16)  # [T_full, D] row layout
    T = h_aug.shape[0] // N_CORES
    n_groups = T // 512
    ht8 = np.ascontiguousarray(
        h_aug.astype(e4)
        .reshape(N_CORES, n_groups, 512, KD, 128)
        .transpose(0, 1, 4, 3, 2)
        .reshape(N_CORES, n_groups * 128, KD * 512)
    )

    Wdg = gam[:, None] * Wd  # [D, DB]
    wd8 = np.ascontiguousarray(
        (WD_SCALE * Wdg).astype(e4)
        .reshape(KD, 128, DB).transpose(1, 0, 2).reshape(128, KD * DB)
    )
    q_h = np.ascontiguousarray(
        ((WD_SCALE / CH) * Wdg.sum(axis=0)).astype(bf).reshape(1, DB)
    )
    b1_h = np.ascontiguousarray(
        (bet @ Wdg + bd).astype(np.float32).reshape(KB, 128).T
    )  # [128, KB]
    cw = CI * CH * alpha / (1.0 - alpha)
    wu8 = np.ascontiguousarray(
        (cw * Wu).astype(e4)
        .reshape(KB, 128, D).transpose(1, 0, 2).reshape(128, KB * D)
    )
    ident = (CI * np.eye(128)).astype(np.float16)
    return hs16, ht8, wd8, wu8, q_h, b1_h, ident, alpha


_cached = {}


def kernel(
    hidden,
    ln_gamma,
    ln_beta,
    W_down,
    b_down,
    W_up,
    b_up,
    alpha,
    layer_idx=None,
    **_unused,
):
    hs16, ht8, wd8, wu8, q_h, b1_h, ident, alpha_f = prep_host_inputs(
        hidden, ln_gamma, ln_beta, W_down, b_down, W_up, b_up, alpha
    )
    T = hs16.shape[0] // N_CORES
    key = (T, alpha_f)
    if key not in _cached:
        _cached[key] = build_kernel(T, alpha_f)
    nc = _cached[key]

    h_shards = hs16.reshape(N_CORES, T, D)
    in_maps = [
        {
            "h": np.ascontiguousarray(h_shards[c]),
            "ht": ht8[c],
            "wd": wd8,
            "wu": wu8,
            "qs": q_h,
            "b1": b1_h,
            "ident": ident,
        }
        for c in range(N_CORES)
    ]
    res = run_bass_kernel_spmd(nc, in_maps, list(range(N_CORES)))
    global _last_results
    _last_results = res
    out = np.concatenate([r["o"] for r in res.results], axis=0)
    return out.reshape(np.asarray(hidden).shape).astype(np.float32)


_last_results = None
